# revision 42
# baseline (speedup 1.0000x reference)
# Multi-head attention kernel for Trainium2, sharded over 8 NeuronCores.
#
# Sharding: core = (batch b, query-half qh, head-half hh). Each core handles
# 6 heads (3 head-pairs) x 1024 queries of one batch, computing K/V
# projections only for its own 6 heads (no cross-core recompute). The output
# projection is a PARTIAL sum over the core's 6 heads; the two head-half
# partners' partials are summed on the HOST during assembly (exact fp32 add),
# so no on-chip collective is needed.
#
# The per-core kernel is built around the ScalarE exp stream (96 x
# [128,1024] activations, ~107us), the single largest engine span; all PE
# matmul work (~110us) is software-pipelined underneath it:
#   - attention runs as 6 units (head-pair x query-512-chunk) x 16 key
#     tiles: scores (2 heads row-packed in PE quadrants, K=64) -> exp ->
#     PV with a ones-column in v producing softmax denominators for free.
#     PV trails the exp stream by PVT steps so projection fillers never
#     delay the next scores.
#   - q/k/v projections are emitted as (deadline, not_before) filler chunks
#     drained into the attention steps; inputs stream via three DMA queues
#     (sync/scalar/gpsimd) in need order, with the first [wq|xq] / [wk|xk]
#     slices host-concatenated into single large "head" DMAs.
#   - per-unit softmax normalization (deferred into the next unit): staging
#     copy, denominators partition-shifted 64->0, single-op approx fast
#     reciprocal (only correct at partition 0!), bf16 K=1 broadcast matmul,
#     DVE multiply.
#   - output projection: pairs 0+1 and bias pre-accumulated into SBUF
#     during later units; only pair 2's two matmuls + one add + DMA per
#     128-query tile remain at the end.
# PSUM budget: scores 2x[128,2,512] (4 banks) + PV accumulators 2x[65,512]
# (2 banks) + projection scratch 2x[128,512] (2 banks) = 8 banks exactly.

import numpy as np
import os
from contextlib import ExitStack

_DEBUG = os.environ.get('KDEBUG', '0') == '1'

import concourse.bass as bass
import concourse.mybir as mybir
import concourse.tile as tile
from concourse import bacc
from concourse.bass_utils import run_bass_kernel_spmd

F32 = mybir.dt.float32
BF16 = mybir.dt.bfloat16
P = 128
E = 768
S = 2048
B = 2
H = 12
D = 64
NCORES = 8
EC = E // P        # 6 e-chunks (contraction over hidden)
KT = S // P        # 16 key tiles
HL = 6             # heads per core
MT = HL * D // P   # 3 m-tiles (head pairs) per core
QB = 1024          # queries per core
QC = 2             # query 512-chunks per core
NC4 = S // 512     # 4 n-slices of k^T


def build_nc():
    nc = bacc.Bacc("TRN2", debug=False)

    # DRAM I/O (per-core shapes; same NEFF on all 8 cores)
    # all inputs pre-arranged on host into on-chip [128-partition, ...] layout
    # so every DMA is a contiguous full-bandwidth copy
    # critical-path bundles: [wq-mt0 | xq-qc0] and [wk-mt0 | xk-n0] land as
    # single large DMAs so the exp stream can start ASAP
    qhead = nc.dram_tensor("qhead", (P, EC * P + EC * 512), BF16, kind="ExternalInput")
    khead = nc.dram_tensor("khead", (P, EC * P + EC * 512), BF16, kind="ExternalInput")
    xq1 = nc.dram_tensor("xq1", (P, EC * 512), BF16, kind="ExternalInput")
    xkr = nc.dram_tensor("xkr", (NC4 - 1, P, EC * 512), BF16, kind="ExternalInput")
    xv = nc.dram_tensor("xv", (KT, P, EC * P), BF16, kind="ExternalInput")
    wqr = nc.dram_tensor("wqr", (MT - 1, P, EC * P), BF16, kind="ExternalInput")
    wkr = nc.dram_tensor("wkr", (MT - 1, P, EC * P), BF16, kind="ExternalInput")
    wv = nc.dram_tensor("wv", (P, EC * HL * D), BF16, kind="ExternalInput")
    wo = nc.dram_tensor("wo", (P, MT * E), BF16, kind="ExternalInput")
    bq = nc.dram_tensor("bq", (P, MT), F32, kind="ExternalInput")     # per-partition bias per m-tile
    bk = nc.dram_tensor("bk", (P, MT), F32, kind="ExternalInput")
    bo = nc.dram_tensor("bo", (P, E), F32, kind="ExternalInput")      # partial (bv@Wo [+ bo]), broadcast
    out = nc.dram_tensor("out", (QB, E), F32, kind="ExternalOutput")  # PARTIAL over this core's heads
    dbg = None
    if _DEBUG:
        dbg = nc.dram_tensor("dbg", (D, HL * QC * 512), F32, kind="ExternalOutput")

    with tile.TileContext(nc) as tc:
        with ExitStack() as ctx:
            _emit(ctx, tc, nc, qhead, khead, xq1, xkr, xv, wqr, wkr, wv, wo,
                  bq, bk, bo, out, dbg)
    nc.compile()
    return nc


def _emit(ctx, tc, nc, qhead, khead, xq1, xkr, xv, wqr, wkr, wv, wo,
          bq, bk, bo, out, dbg=None):
    # ---- pools ----
    persist = ctx.enter_context(tc.tile_pool(name="persist", bufs=1))
    wpool = ctx.enter_context(tc.tile_pool(name="wpool", bufs=2))
    xvpool = ctx.enter_context(tc.tile_pool(name="xvpool", bufs=3))
    epool = ctx.enter_context(tc.tile_pool(name="epool", bufs=8))
    outpool = ctx.enter_context(tc.tile_pool(name="outpool", bufs=2))
    # PSUM pools: 4 + 2 + 2 = 8 banks
    psC = ctx.enter_context(tc.tile_pool(name="psC", bufs=2, space="PSUM"))    # scores [128,2,512]
    opool = ctx.enter_context(tc.tile_pool(name="opool", bufs=2, space="PSUM"))  # PV accum [65,512]
    pj = ctx.enter_context(tc.tile_pool(name="pj", bufs=2, space="PSUM"))      # proj scratch [128,512]

    # ---- persistent SBUF tensors ----
    qT = persist.tile([P, MT, QB], BF16)          # q^T [384, 1024]
    kT = persist.tile([P, MT, S], BF16)           # k^T [384, 2048]
    v_sb = persist.tile([P, KT, HL, D + 1], BF16)  # v + ones column per head
    o_all = persist.tile([P, MT, QB], BF16)       # normalized o^T, pairs in partition halves
    o_raw = persist.tile([D, HL, QC, 512], F32)   # staged unnormalized o^T
    dens0 = persist.tile([1, 2, 512], F32)        # denominators relocated to partition 0
    drecf = persist.tile([1, 2, 512], F32)        # fast-reciprocal output (partition 0)
    drecb = persist.tile([1, 2, 512], BF16)       # bf16 copy feeding the bc matmul
    sel0 = persist.tile([1, 512], BF16)           # ones row: bc selector + PE warm-up feed
    opart = persist.tile([P, QB // P, E], F32)    # pair0+pair1+bias partial out
    bcl_sb = persist.tile([D, 2, 512], F32)       # tail broadcast staged via ScalarE
    bq_sb = persist.tile([P, MT], F32)
    bk_sb = persist.tile([P, MT], F32)
    bo_sb = persist.tile([P, E], F32)
    qh_t = persist.tile([P, EC * P + EC * 512], BF16)   # wq-mt0 | xq-qc0
    kh_t = persist.tile([P, EC * P + EC * 512], BF16)   # wk-mt0 | xk-n0
    xq1_t = persist.tile([P, EC, 512], BF16)
    xkr_t = persist.tile([P, NC4 - 1, EC, 512], BF16)
    wqr_t = persist.tile([P, MT - 1, EC, P], BF16)
    wkr_t = persist.tile([P, MT - 1, EC, P], BF16)
    WOFF = EC * P

    # ---- DMAs: sync carries the q-chain bundle + late keys, scalar the
    # k-chain bundle + wv + early xv, gpsimd only small/late-needed items.
    nc.sync.dma_start(qh_t[:], qhead[:])
    nc.scalar.dma_start(kh_t[:], khead[:])
    wv_t = wpool.tile([P, EC, HL * D], BF16, tag="w")
    nc.gpsimd.dma_start(wv_t[:].rearrange("p a b -> p (a b)"), wv[:])
    nc.gpsimd.dma_start(bq_sb[:], bq[:])
    nc.gpsimd.dma_start(bk_sb[:], bk[:])
    nc.sync.dma_start(xkr_t[:, 0, :, :].rearrange("p a b -> p (a b)"), xkr[0])
    nc.scalar.dma_start(xkr_t[:, 1, :, :].rearrange("p a b -> p (a b)"), xkr[1])
    nc.scalar.dma_start(xkr_t[:, 2, :, :].rearrange("p a b -> p (a b)"), xkr[2])
    for mt in range(1, MT):
        nc.gpsimd.dma_start(wqr_t[:, mt - 1, :, :].rearrange("p a b -> p (a b)"), wqr[mt - 1])
        nc.gpsimd.dma_start(wkr_t[:, mt - 1, :, :].rearrange("p a b -> p (a b)"), wkr[mt - 1])

    def emit_xq1():
        nc.sync.dma_start(xq1_t[:].rearrange("p a b -> p (a b)"), xq1[:])

    def emit_bo():
        nc.sync.dma_start(bo_sb[:], bo[:])

    # constants: ones column for denominators, selector row for broadcast
    nc.vector.memset(v_sb[:, :, :, D], 1.0)
    nc.vector.memset(sel0[:], 1.0)

    # ---- PE warm-up: ~5us of dummy matmuls on memset data while the first
    # input DMAs are in flight, so the HAM clock-gate reaches 2.4GHz before
    # real work lands (cold MMs run at 1.2GHz otherwise)
    warm = pj.tile([D, 512], F32, tag="pj", name="warm")
    for i in range(12):
        nc.tensor.matmul(warm[:], sel0[:, 0:D], sel0[:],
                         start=(i == 0), stop=(i == 11))
    nc.vector.tensor_copy(o_raw[0:D, 0, 0, :], warm[:])  # consume (overwritten later)

    # ---- projection emitters (interleaved as filler work) ----
    def emit_q(mt, qc):
        ps = pj.tile([P, 512], F32, tag="pj")
        for ec in range(EC):
            lhsT = qh_t[:, ec * P:(ec + 1) * P] if mt == 0 else wqr_t[:, mt - 1, ec, :]
            rhs = (qh_t[:, WOFF + ec * 512:WOFF + (ec + 1) * 512] if qc == 0
                   else xq1_t[:, ec, :])
            nc.tensor.matmul(ps[:], lhsT, rhs, start=(ec == 0), stop=(ec == EC - 1))
        nc.vector.tensor_scalar_add(qT[:, mt, qc * 512:(qc + 1) * 512], ps[:], bq_sb[:, mt:mt + 1])

    def emit_k(mt, n4):
        ps = pj.tile([P, 512], F32, tag="pj")
        for ec in range(EC):
            lhsT = kh_t[:, ec * P:(ec + 1) * P] if mt == 0 else wkr_t[:, mt - 1, ec, :]
            rhs = (kh_t[:, WOFF + ec * 512:WOFF + (ec + 1) * 512] if n4 == 0
                   else xkr_t[:, n4 - 1, ec, :])
            nc.tensor.matmul(ps[:], lhsT, rhs, start=(ec == 0), stop=(ec == EC - 1))
        nc.vector.tensor_scalar_add(kT[:, mt, n4 * 512:(n4 + 1) * 512], ps[:], bk_sb[:, mt:mt + 1])

    xv_tiles = {}

    def emit_v_dma(kt):
        xv_t = xvpool.tile([P, EC, P], BF16, tag="xv")
        eng = nc.scalar if 6 <= kt < 12 else (nc.gpsimd if kt >= 14 else nc.sync)
        eng.dma_start(xv_t[:].rearrange("p a b -> p (a b)"), xv[kt])
        xv_tiles[kt] = xv_t

    def emit_v(kt):
        xv_t = xv_tiles.pop(kt)
        ps = pj.tile([P, 512], F32, tag="pj")  # only 384 used
        for ec in range(EC):
            nc.tensor.matmul(ps[:, 0:HL * D], xv_t[:, ec, :], wv_t[:, ec, :],
                             start=(ec == 0), stop=(ec == EC - 1))
        nc.vector.tensor_copy(v_sb[:, kt, :, 0:D],
                              ps[:, 0:HL * D].rearrange("p (h d) -> p h d", d=D))

    # Deadline-ordered filler queue: (deadline_step, emit_fn). Steps run
    # 0..95 (6 units x 16 key tiles). Forced at deadline; otherwise drained
    # at DRAIN_BUDGET items/step to spread PE work under the exp stream.
    fillers = []
    # (deadline, not_before, fn): forced at deadline; budget-drained only
    # once `not_before` is reached (so PE work never lands ahead of its DMA)
    dma_dl = {0: 0, 1: 0, 2: 1, 3: 2, 4: 3, 5: 4, 6: 5, 7: 5,
              8: 7, 9: 8, 10: 9, 11: 10, 12: 11, 13: 12, 14: 13, 15: 14}
    for kt in range(KT):
        fillers.append((dma_dl[kt], 0, lambda kt=kt: emit_v_dma(kt)))
    fillers.append((6, 6, emit_xq1))
    fillers.append((40, 36, emit_bo))
    for n4 in range(1, NC4):
        fillers.append((4 * n4 - 1, 4 * n4 - 2, lambda n4=n4: emit_k(0, n4)))
    # v-projection MMs: must be emitted before PV(unit0, kt), which trails
    # the exp stream by PVT steps (Tile deps follow emission order)
    for kt in range(KT):
        fillers.append((max(kt + 3, 4), max(kt + 1, 4), lambda kt=kt: emit_v(kt)))
    # pair-major unit order: u1=(p0,qc1)@16, u2=(p1,qc0)@32, u4=(p2,qc0)@64
    fillers.append((12, 8, lambda: emit_q(0, 1)))
    fillers.append((21, 16, lambda: emit_q(1, 0)))
    for n4 in range(NC4):
        fillers.append((22 + n4, 17 + n4, lambda n4=n4: emit_k(1, n4)))
    fillers.append((44, 40, lambda: emit_q(1, 1)))
    fillers.append((52, 47, lambda: emit_q(2, 0)))
    for n4 in range(NC4):
        fillers.append((53 + n4, 48 + n4, lambda n4=n4: emit_k(2, n4)))
    fillers.append((74, 70, lambda: emit_q(2, 1)))
    fillers.sort(key=lambda t: t[0])
    fidx = [0]

    def drain_fillers(step, budget):
        n = 0
        while fidx[0] < len(fillers) and (
                fillers[fidx[0]][0] <= step
                or (n < budget and fillers[fidx[0]][1] <= step)):
            fillers[fidx[0]][2]()
            fidx[0] += 1
            n += 1

    # ---- prologue: minimal work before the exp stream starts ----
    emit_q(0, 0)
    emit_k(0, 0)

    # ---- attention units: qc-major (all qc0 pairs first) so the qc0
    # output projection can run interleaved into the qc1 units.
    # PV trails the exp stream by PVT steps so v-projection fillers keep
    # lower PE priority than scores and the exp cadence never breaks.
    def make_norm(pair, qc, o_ps):
        def norm():
            for i in range(2):
                h = 2 * pair + i
                nc.vector.tensor_copy(o_raw[:, h, qc, :], o_ps[i][0:D, :])  # stage; frees PSUM
                # denom row partition-shifted 64 -> 0 (approx_fast recip is
                # only correct at base partition 0)
                nc.vector.tensor_copy(dens0[:, i, :], o_ps[i][D:D + 1, :])
            for i in range(2):
                nc.vector.reciprocal_approx_fast(drecf[:, i, :], dens0[:, i, :])
            nc.vector.tensor_copy(drecb[:], drecf[:])
            for i in range(2):
                h = 2 * pair + i
                bc = pj.tile([D, 512], F32, tag="pj", name=f"bc{i}")
                nc.tensor.matmul(bc[:], sel0[:, 0:D], drecb[:, i, :],
                                 start=True, stop=True)
                nc.vector.tensor_tensor(o_all[i * D:(i + 1) * D, pair, qc * 512:(qc + 1) * 512],
                                        o_raw[:, h, qc, :], bc[:], mybir.AluOpType.mult)
        return norm

    wo_t = wpool.tile([P, MT, E], BF16, tag="w")

    def emit_wo():
        nc.scalar.dma_start(wo_t[:].rearrange("p a b -> p (a b)"), wo[:])
    fillers.append((34, 30, emit_wo))
    fillers.sort(key=lambda t: t[0])

    def emit_opart(st8):
        # pairs 0,1 + bias accumulated into SBUF; pair 2 lands later
        op1 = pj.tile([P, 512], F32, tag="pj", name="op1")
        op2 = pj.tile([P, 256], F32, tag="pj", name="op2")
        for pair in (0, 1):
            first, last = (pair == 0), (pair == 1)
            nc.tensor.matmul(op1[:], o_all[:, pair, st8 * P:(st8 + 1) * P],
                             wo_t[:, pair, 0:512], start=first, stop=last)
            nc.tensor.matmul(op2[:], o_all[:, pair, st8 * P:(st8 + 1) * P],
                             wo_t[:, pair, 512:768], start=first, stop=last)
        nc.vector.tensor_tensor(opart[:, st8, 0:512], op1[:], bo_sb[:, 0:512],
                                mybir.AluOpType.add)
        nc.vector.tensor_tensor(opart[:, st8, 512:768], op2[:], bo_sb[:, 512:768],
                                mybir.AluOpType.add)

    def emit_ofin(st8, pool=None):
        pool = pool or pj
        tg = "pj" if pool is pj else "o"
        op1 = pool.tile([P, 512], F32, tag=tg, name="op1")
        op2 = pool.tile([P, 256], F32, tag=tg, name="op2")
        nc.tensor.matmul(op1[:], o_all[:, 2, st8 * P:(st8 + 1) * P],
                         wo_t[:, 2, 0:512], start=True, stop=True)
        nc.tensor.matmul(op2[:], o_all[:, 2, st8 * P:(st8 + 1) * P],
                         wo_t[:, 2, 512:768], start=True, stop=True)
        out_sb = outpool.tile([P, E], F32, tag="osb")
        nc.vector.tensor_tensor(out_sb[:, 0:512], opart[:, st8, 0:512], op1[:],
                                mybir.AluOpType.add)
        nc.vector.tensor_tensor(out_sb[:, 512:768], opart[:, st8, 512:768], op2[:],
                                mybir.AluOpType.add)
        eng = (nc.sync, nc.scalar, nc.gpsimd)[st8 % 3]
        eng.dma_start(out[st8 * P:(st8 + 1) * P, :], out_sb[:])

    PVT = 4
    units = [(0, 0), (0, 1), (1, 0), (1, 1), (2, 0), (2, 1)]  # (pair, qc)
    # partials (pairs 0,1) during u4; pair-2 finishes for qc0 during u5
    opart_at = {(3, 6): 0, (3, 9): 1, (3, 12): 2, (3, 15): 3,
                (4, 8): 4, (4, 10): 5, (4, 12): 6, (4, 14): 7}
    ofin_at = {(5, 6): 0, (5, 8): 1, (5, 10): 2, (5, 12): 3}
    pending_norm = None
    pending_flush = []

    def flush_one():
        for _ in range(2):
            if pending_flush:
                pending_flush.pop(0)()

    for u, (pair, qc) in enumerate(units):
        o_ps = {i: opool.tile([D + 1, 512], F32, tag="o", name=f"o{i}") for i in range(2)}
        exq = []
        for kt in range(KT):
            drain_fillers(u * KT + kt, 2)
            flush_one()
            if kt == 6 and pending_norm is not None:
                pending_norm()
                pending_norm = None
            if (u, kt) in opart_at:
                emit_opart(opart_at[(u, kt)])
            if (u, kt) in ofin_at:
                emit_ofin(ofin_at[(u, kt)])
            st = psC.tile([P, 2, 512], F32, tag="sc")
            for i in range(2):
                po = D * i
                nc.tensor.matmul(st[:, i, :],
                                 kT[po:po + D, pair, kt * P:(kt + 1) * P],
                                 qT[po:po + D, pair, qc * 512:(qc + 1) * 512],
                                 start=True, stop=True)
            ex = epool.tile([P, 2, 512], BF16, tag="ex")
            nc.scalar.activation(ex[:, :, :], st[:, :, :], mybir.ActivationFunctionType.Exp)
            exq.append(ex)
            trail = PVT if u < len(units) - 1 else 0
            if kt >= trail:
                kk = kt - trail
                for i in range(2):
                    nc.tensor.matmul(o_ps[i][:, :], v_sb[:, kk, 2 * pair + i, :],
                                     exq[kk][:, i, :], start=(kk == 0),
                                     stop=(kk == KT - 1))

        def make_flush(pair, qc, o_ps, exq, kk):
            def f():
                for i in range(2):
                    nc.tensor.matmul(o_ps[i][:, :], v_sb[:, kk, 2 * pair + i, :],
                                     exq[kk][:, i, :], start=False, stop=(kk == KT - 1))
            return f
        if u < len(units) - 1:
            pending_flush = [make_flush(pair, qc, o_ps, exq, kk)
                             for kk in range(KT - PVT, KT)]
            pending_norm = make_norm(pair, qc, o_ps)
    last_ops = o_ps

    if dbg is not None:
        nc.sync.dma_start(dbg[:, :], o_raw[:].rearrange("p a b c -> p (a b c)"))

    lp, lqc = units[-1]
    warm2 = pj.tile([D, 512], F32, tag="pj", name="warm2")
    for i in range(10):
        nc.tensor.matmul(warm2[:], sel0[:, 0:D], sel0[:],
                         start=(i == 0), stop=(i == 9))
    nc.vector.tensor_copy(o_raw[0:D, 0, 0, :], warm2[:])
    for i in range(2):
        nc.vector.tensor_copy(dens0[:, i, :], last_ops[i][D:D + 1, :])
    for i in range(2):
        nc.vector.reciprocal_approx_fast(drecf[:, i, :], dens0[:, i, :])
    nc.vector.tensor_copy(drecb[:], drecf[:])
    for i in range(2):
        bc = pj.tile([D, 512], F32, tag="pj", name=f"bct{i}")
        nc.tensor.matmul(bc[:], sel0[:, 0:D], drecb[:, i, :], start=True, stop=True)
        nc.scalar.copy(bcl_sb[:, i, :], bc[:])
        nc.vector.tensor_tensor(o_all[i * D:(i + 1) * D, lp, lqc * 512:(lqc + 1) * 512],
                                last_ops[i][0:D, :], bcl_sb[:, i, :],
                                mybir.AluOpType.mult)
    for st8 in range(4, QB // P):
        emit_ofin(st8, pool=(pj if st8 % 2 == 0 else opool))


_NC_CACHE = None


def _get_nc():
    global _NC_CACHE
    if _NC_CACHE is None:
        _NC_CACHE = build_nc()
    return _NC_CACHE


def make_in_maps(query, key_, value, Wq, bq, Wk, bk, Wv, bv, Wo, bo):
    """Host-side sharding + layout prep. Returns list of 8 input dicts."""
    import ml_dtypes
    BF = ml_dtypes.bfloat16
    query = np.asarray(query, dtype=np.float32)
    key_ = np.asarray(key_, dtype=np.float32)
    value = np.asarray(value, dtype=np.float32)
    scale = np.float32(1.0 / np.sqrt(np.float32(D)))

    Wq = np.asarray(Wq, np.float32)
    Wk = np.asarray(Wk, np.float32)
    Wv = np.asarray(Wv, np.float32)
    Wo = np.asarray(Wo, np.float32)
    bq_f = np.asarray(bq, np.float32)
    bk_f = np.asarray(bk, np.float32)
    bv_f = np.asarray(bv, np.float32)
    bo_f = np.asarray(bo, np.float32)

    def pem(a):
        # [E, M] -> [128p, EC, M] -> flat [128, EC*M]
        E_, m = a.shape
        return np.ascontiguousarray(a.reshape(EC, P, m).transpose(1, 0, 2).reshape(P, EC * m))

    def xslices(a, width):
        # [E, S] -> [S//width, 128, EC*width]
        E_, s = a.shape
        n = s // width
        r = a.reshape(EC, P, n, width).transpose(2, 1, 0, 3)
        return np.ascontiguousarray(r.reshape(n, P, EC * width))

    xk_t = [xslices(key_[b].T, 512).astype(BF) for b in range(B)]
    xv_t = [xslices(value[b].T, P).astype(BF) for b in range(B)]
    xq_t = {}
    for b in range(B):
        for qh in range(2):
            xq_t[(b, qh)] = xslices(query[b, qh * QB:(qh + 1) * QB, :].T, 512).astype(BF)

    per_hh = {}
    for hh in range(2):
        hs = slice(hh * HL, (hh + 1) * HL)
        wq_f = np.transpose(Wq[hs], (1, 0, 2)).reshape(E, HL * D) * scale
        wk_f = np.transpose(Wk[hs], (1, 0, 2)).reshape(E, HL * D)
        wv_f = np.transpose(Wv[hs], (1, 0, 2)).reshape(E, HL * D)
        wo_f = Wo[hh * HL * D:(hh + 1) * HL * D, :]
        # wq/wk: per-m-tile chunks [MT, 128, EC*128]
        wq_c = np.stack([pem(wq_f[:, mt * P:(mt + 1) * P]) for mt in range(MT)]).astype(BF)
        wk_c = np.stack([pem(wk_f[:, mt * P:(mt + 1) * P]) for mt in range(MT)]).astype(BF)
        wv_c = pem(wv_f).astype(BF)
        # wo: [384, 768] -> [128, MT*768], partition p holds row mt*128+p
        wo_c = np.ascontiguousarray(
            wo_f.reshape(MT, P, E).transpose(1, 0, 2).reshape(P, MT * E)).astype(BF)
        bq_p = (bq_f[hs].reshape(HL * D) * scale).reshape(MT, P).T.copy()
        bk_p = bk_f[hs].reshape(HL * D).reshape(MT, P).T.copy()
        # v-bias folded through this core's Wo rows; bo itself only on hh=0
        bo_eff = bv_f[hs].reshape(HL * D) @ wo_f
        if hh == 0:
            bo_eff = bo_eff + bo_f
        per_hh[hh] = dict(
            wq0=wq_c[0], wqr=np.ascontiguousarray(wq_c[1:]),
            wk0=wk_c[0], wkr=np.ascontiguousarray(wk_c[1:]),
            wv=wv_c, wo=wo_c, bq=bq_p, bk=bk_p,
            bo=np.tile(bo_eff.reshape(1, E), (P, 1)).astype(np.float32).copy(),
        )

    in_maps = []
    for core in range(NCORES):
        b, qh, hh = core // 4, (core // 2) % 2, core % 2
        p = per_hh[hh]
        m = dict(
            qhead=np.ascontiguousarray(np.concatenate([p["wq0"], xq_t[(b, qh)][0]], axis=1)),
            khead=np.ascontiguousarray(np.concatenate([p["wk0"], xk_t[b][0]], axis=1)),
            xq1=xq_t[(b, qh)][1],
            xkr=np.ascontiguousarray(xk_t[b][1:]),
            wqr=p["wqr"], wkr=p["wkr"], wv=p["wv"], wo=p["wo"],
            bq=p["bq"], bk=p["bk"], bo=p["bo"],
            xv=xv_t[b],
        )
        in_maps.append(m)
    return in_maps


def assemble(results):
    outp = np.empty((B, S, E), dtype=np.float32)
    for b in range(B):
        for qh in range(2):
            c0 = b * 4 + qh * 2
            outp[b, qh * QB:(qh + 1) * QB, :] = results[c0]["out"] + results[c0 + 1]["out"]
    return outp


def kernel(query, key_, value, Wq, bq, Wk, bk, Wv, bv, Wo, bo):
    nc = _get_nc()
    in_maps = make_in_maps(query, key_, value, Wq, bq, Wk, bk, Wv, bv, Wo, bo)
    res = run_bass_kernel_spmd(nc, in_maps, core_ids=list(range(NCORES)))
    return assemble(res.results)


# revision 43
# speedup vs baseline: 1.1602x; 1.1602x over previous
# Multi-head attention kernel for Trainium2, sharded over 8 NeuronCores.
#
# Sharding: core = (batch b, query-half qh, head-half hh). Each core handles
# 6 heads (3 head-pairs) x 1024 queries of one batch, computing K/V
# projections only for its own 6 heads (no cross-core recompute). The output
# projection is a PARTIAL sum over the core's 6 heads; the two head-half
# partners' partials are summed on the HOST during assembly (exact fp32 add),
# so no on-chip collective is needed.
#
# The per-core kernel is built around the ScalarE exp stream (96 x
# [128,1024] activations, ~107us), the single largest engine span; all PE
# matmul work (~110us) is software-pipelined underneath it:
#   - attention runs as 6 units (head-pair x query-512-chunk) x 16 key
#     tiles: scores (2 heads row-packed in PE quadrants, K=64) -> exp ->
#     PV with a ones-column in v producing softmax denominators for free.
#     PV trails the exp stream by PVT steps so projection fillers never
#     delay the next scores.
#   - q/k/v projections are emitted as (deadline, not_before) filler chunks
#     drained into the attention steps; inputs stream via three DMA queues
#     (sync/scalar/gpsimd) in need order, with the first [wq|xq] / [wk|xk]
#     slices host-concatenated into single large "head" DMAs.
#   - per-unit softmax normalization (deferred into the next unit): staging
#     copy, denominators partition-shifted 64->0, single-op approx fast
#     reciprocal (only correct at partition 0!), bf16 K=1 broadcast matmul,
#     DVE multiply.
#   - output projection: pairs 0+1 and bias pre-accumulated into SBUF
#     during later units; only pair 2's two matmuls + one add + DMA per
#     128-query tile remain at the end.
# PSUM budget: scores 2x[128,2,512] (4 banks) + PV accumulators 2x[65,512]
# (2 banks) + projection scratch 2x[128,512] (2 banks) = 8 banks exactly.

import numpy as np
import os
from contextlib import ExitStack

_DEBUG = os.environ.get('KDEBUG', '0') == '1'

import concourse.bass as bass
import concourse.mybir as mybir
import concourse.tile as tile
from concourse import bacc
from concourse.bass_utils import run_bass_kernel_spmd

F32 = mybir.dt.float32
BF16 = mybir.dt.bfloat16
P = 128
E = 768
S = 2048
B = 2
H = 12
D = 64
NCORES = 8
EC = E // P        # 6 e-chunks (contraction over hidden)
KT = S // P        # 16 key tiles
HL = 6             # heads per core
MT = HL * D // P   # 3 m-tiles (head pairs) per core
QB = 1024          # queries per core
QC = 2             # query 512-chunks per core
NC4 = S // 512     # 4 n-slices of k^T


def build_nc():
    nc = bacc.Bacc("TRN2", debug=False)

    # DRAM I/O (per-core shapes; same NEFF on all 8 cores)
    # all inputs pre-arranged on host into on-chip [128-partition, ...] layout
    # so every DMA is a contiguous full-bandwidth copy
    # critical-path bundles: [wq-mt0 | xq-qc0] and [wk-mt0 | xk-n0] land as
    # single large DMAs so the exp stream can start ASAP
    qhead = nc.dram_tensor("qhead", (P, EC * P + EC * 512), BF16, kind="ExternalInput")
    khead = nc.dram_tensor("khead", (P, EC * P + EC * 512), BF16, kind="ExternalInput")
    xq1 = nc.dram_tensor("xq1", (P, EC * 512), BF16, kind="ExternalInput")
    xkr = nc.dram_tensor("xkr", (NC4 - 1, P, EC * 512), BF16, kind="ExternalInput")
    xv = nc.dram_tensor("xv", (KT, P, EC * P), BF16, kind="ExternalInput")
    wqr = nc.dram_tensor("wqr", (MT - 1, P, EC * P), BF16, kind="ExternalInput")
    wkr = nc.dram_tensor("wkr", (MT - 1, P, EC * P), BF16, kind="ExternalInput")
    wv = nc.dram_tensor("wv", (P, EC * HL * D), BF16, kind="ExternalInput")
    wo = nc.dram_tensor("wo", (P, MT * E), BF16, kind="ExternalInput")
    bq = nc.dram_tensor("bq", (P, MT), F32, kind="ExternalInput")     # per-partition bias per m-tile
    bk = nc.dram_tensor("bk", (P, MT), F32, kind="ExternalInput")
    bo = nc.dram_tensor("bo", (P, E), F32, kind="ExternalInput")      # partial (bv@Wo [+ bo]), broadcast
    out = nc.dram_tensor("out", (QB, E), F32, kind="ExternalOutput")  # PARTIAL over this core's heads
    dbg = None
    if _DEBUG:
        dbg = nc.dram_tensor("dbg", (D, HL * QC * 512), F32, kind="ExternalOutput")

    with tile.TileContext(nc) as tc:
        with ExitStack() as ctx:
            _emit(ctx, tc, nc, qhead, khead, xq1, xkr, xv, wqr, wkr, wv, wo,
                  bq, bk, bo, out, dbg)
    nc.compile()
    return nc


def _emit(ctx, tc, nc, qhead, khead, xq1, xkr, xv, wqr, wkr, wv, wo,
          bq, bk, bo, out, dbg=None):
    # ---- pools ----
    persist = ctx.enter_context(tc.tile_pool(name="persist", bufs=1))
    wpool = ctx.enter_context(tc.tile_pool(name="wpool", bufs=2))
    xvpool = ctx.enter_context(tc.tile_pool(name="xvpool", bufs=3))
    epool = ctx.enter_context(tc.tile_pool(name="epool", bufs=8))
    outpool = ctx.enter_context(tc.tile_pool(name="outpool", bufs=2))
    # PSUM pools: 4 + 2 + 2 = 8 banks
    psC = ctx.enter_context(tc.tile_pool(name="psC", bufs=2, space="PSUM"))    # scores [128,2,512]
    opool = ctx.enter_context(tc.tile_pool(name="opool", bufs=2, space="PSUM"))  # PV accum [65,512]
    pj = ctx.enter_context(tc.tile_pool(name="pj", bufs=2, space="PSUM"))      # proj scratch [128,512]

    # ---- persistent SBUF tensors ----
    qT = persist.tile([P, MT, QB], BF16)          # q^T [384, 1024]
    kT = persist.tile([P, MT, S], BF16)           # k^T [384, 2048]
    v_sb = persist.tile([P, KT, HL, D + 1], BF16)  # v + ones column per head
    o_all = persist.tile([P, MT, QB], BF16)       # normalized o^T, pairs in partition halves
    o_raw = persist.tile([D, HL, QC, 512], F32)   # staged unnormalized o^T
    dens0 = persist.tile([1, 2, 512], F32)        # denominators relocated to partition 0
    drecf = persist.tile([1, 2, 512], F32)        # fast-reciprocal output (partition 0)
    drecb = persist.tile([1, 2, 512], BF16)       # bf16 copy feeding the bc matmul
    sel0 = persist.tile([1, 512], BF16)           # ones row: bc selector + PE warm-up feed
    opart = persist.tile([P, QB // P, E], F32)    # pair0+pair1+bias partial out
    bcl_sb = persist.tile([D, 2, 512], F32)       # tail broadcast staged via ScalarE
    bq_sb = persist.tile([P, MT], F32)
    bk_sb = persist.tile([P, MT], F32)
    bo_sb = persist.tile([P, E], F32)
    qh_t = persist.tile([P, EC * P + EC * 512], BF16)   # wq-mt0 | xq-qc0
    kh_t = persist.tile([P, EC * P + EC * 512], BF16)   # wk-mt0 | xk-n0
    xq1_t = persist.tile([P, EC, 512], BF16)
    xkr_t = persist.tile([P, NC4 - 1, EC, 512], BF16)
    wqr_t = persist.tile([P, MT - 1, EC, P], BF16)
    wkr_t = persist.tile([P, MT - 1, EC, P], BF16)
    WOFF = EC * P

    # ---- DMAs: sync carries the q-chain bundle + late keys, scalar the
    # k-chain bundle + wv + early xv, gpsimd only small/late-needed items.
    nc.sync.dma_start(qh_t[:], qhead[:])
    nc.scalar.dma_start(kh_t[:], khead[:])
    wv_t = wpool.tile([P, EC, HL * D], BF16, tag="w")
    nc.gpsimd.dma_start(wv_t[:].rearrange("p a b -> p (a b)"), wv[:])
    nc.gpsimd.dma_start(bq_sb[:], bq[:])
    nc.gpsimd.dma_start(bk_sb[:], bk[:])
    nc.sync.dma_start(xkr_t[:, 0, :, :].rearrange("p a b -> p (a b)"), xkr[0])
    nc.scalar.dma_start(xkr_t[:, 1, :, :].rearrange("p a b -> p (a b)"), xkr[1])
    nc.scalar.dma_start(xkr_t[:, 2, :, :].rearrange("p a b -> p (a b)"), xkr[2])
    for mt in range(1, MT):
        nc.gpsimd.dma_start(wqr_t[:, mt - 1, :, :].rearrange("p a b -> p (a b)"), wqr[mt - 1])
        nc.gpsimd.dma_start(wkr_t[:, mt - 1, :, :].rearrange("p a b -> p (a b)"), wkr[mt - 1])

    def emit_xq1():
        nc.sync.dma_start(xq1_t[:].rearrange("p a b -> p (a b)"), xq1[:])

    def emit_bo():
        nc.sync.dma_start(bo_sb[:], bo[:])

    # constants: ones column for denominators, selector row for broadcast
    nc.vector.memset(v_sb[:, :, :, D], 1.0)
    nc.vector.memset(sel0[:], 1.0)

    # ---- PE warm-up: ~5us of dummy matmuls on memset data while the first
    # input DMAs are in flight, so the HAM clock-gate reaches 2.4GHz before
    # real work lands (cold MMs run at 1.2GHz otherwise)
    warm = pj.tile([D, 512], F32, tag="pj", name="warm")
    for i in range(12):
        nc.tensor.matmul(warm[:], sel0[:, 0:D], sel0[:],
                         start=(i == 0), stop=(i == 11))
    nc.vector.tensor_copy(o_raw[0:D, 0, 0, :], warm[:])  # consume (overwritten later)

    # ---- projection emitters (interleaved as filler work) ----
    def emit_q(mt, qc):
        ps = pj.tile([P, 512], F32, tag="pj")
        for ec in range(EC):
            lhsT = qh_t[:, ec * P:(ec + 1) * P] if mt == 0 else wqr_t[:, mt - 1, ec, :]
            rhs = (qh_t[:, WOFF + ec * 512:WOFF + (ec + 1) * 512] if qc == 0
                   else xq1_t[:, ec, :])
            nc.tensor.matmul(ps[:], lhsT, rhs, start=(ec == 0), stop=(ec == EC - 1))
        nc.vector.tensor_scalar_add(qT[:, mt, qc * 512:(qc + 1) * 512], ps[:], bq_sb[:, mt:mt + 1])

    def emit_k(mt, n4):
        ps = pj.tile([P, 512], F32, tag="pj")
        for ec in range(EC):
            lhsT = kh_t[:, ec * P:(ec + 1) * P] if mt == 0 else wkr_t[:, mt - 1, ec, :]
            rhs = (kh_t[:, WOFF + ec * 512:WOFF + (ec + 1) * 512] if n4 == 0
                   else xkr_t[:, n4 - 1, ec, :])
            nc.tensor.matmul(ps[:], lhsT, rhs, start=(ec == 0), stop=(ec == EC - 1))
        nc.vector.tensor_scalar_add(kT[:, mt, n4 * 512:(n4 + 1) * 512], ps[:], bk_sb[:, mt:mt + 1])

    xv_tiles = {}

    def emit_v_dma(kt):
        xv_t = xvpool.tile([P, EC, P], BF16, tag="xv")
        eng = nc.scalar if 6 <= kt < 12 else nc.sync
        eng.dma_start(xv_t[:].rearrange("p a b -> p (a b)"), xv[kt])
        xv_tiles[kt] = xv_t

    def emit_v(kt):
        xv_t = xv_tiles.pop(kt)
        ps = pj.tile([P, 512], F32, tag="pj")  # only 384 used
        for ec in range(EC):
            nc.tensor.matmul(ps[:, 0:HL * D], xv_t[:, ec, :], wv_t[:, ec, :],
                             start=(ec == 0), stop=(ec == EC - 1))
        nc.vector.tensor_copy(v_sb[:, kt, :, 0:D],
                              ps[:, 0:HL * D].rearrange("p (h d) -> p h d", d=D))

    # Deadline-ordered filler queue: (deadline_step, emit_fn). Steps run
    # 0..95 (6 units x 16 key tiles). Forced at deadline; otherwise drained
    # at DRAIN_BUDGET items/step to spread PE work under the exp stream.
    fillers = []
    # (deadline, not_before, fn): forced at deadline; budget-drained only
    # once `not_before` is reached (so PE work never lands ahead of its DMA)
    dma_dl = {0: 0, 1: 0, 2: 1, 3: 2, 4: 3, 5: 4, 6: 5, 7: 5,
              8: 7, 9: 8, 10: 9, 11: 10, 12: 11, 13: 12, 14: 13, 15: 14}
    for kt in range(KT):
        fillers.append((dma_dl[kt], 0, lambda kt=kt: emit_v_dma(kt)))
    fillers.append((6, 6, emit_xq1))
    fillers.append((40, 36, emit_bo))
    for n4 in range(1, NC4):
        fillers.append((4 * n4 - 1, 4 * n4 - 2, lambda n4=n4: emit_k(0, n4)))
    # v-projection MMs: must be emitted before PV(unit0, kt), which trails
    # the exp stream by PVT steps (Tile deps follow emission order)
    for kt in range(KT):
        fillers.append((max(kt + 3, 4), max(kt + 1, 4), lambda kt=kt: emit_v(kt)))
    # pair-major unit order: u1=(p0,qc1)@16, u2=(p1,qc0)@32, u4=(p2,qc0)@64
    fillers.append((12, 8, lambda: emit_q(0, 1)))
    fillers.append((26, 20, lambda: emit_q(1, 0)))
    for n4 in range(NC4):
        fillers.append((27 + n4, 21 + n4, lambda n4=n4: emit_k(1, n4)))
    fillers.append((44, 40, lambda: emit_q(1, 1)))
    fillers.append((56, 52, lambda: emit_q(2, 0)))
    for n4 in range(NC4):
        fillers.append((57 + n4, 53 + n4, lambda n4=n4: emit_k(2, n4)))
    fillers.append((74, 70, lambda: emit_q(2, 1)))
    fillers.sort(key=lambda t: t[0])
    fidx = [0]

    def drain_fillers(step, budget):
        n = 0
        while fidx[0] < len(fillers) and (
                fillers[fidx[0]][0] <= step
                or (n < budget and fillers[fidx[0]][1] <= step)):
            fillers[fidx[0]][2]()
            fidx[0] += 1
            n += 1

    # ---- prologue: minimal work before the exp stream starts ----
    emit_q(0, 0)
    emit_k(0, 0)

    # ---- attention units: qc-major (all qc0 pairs first) so the qc0
    # output projection can run interleaved into the qc1 units.
    # PV trails the exp stream by PVT steps so v-projection fillers keep
    # lower PE priority than scores and the exp cadence never breaks.
    def make_norm(pair, qc, o_ps):
        def norm():
            for i in range(2):
                h = 2 * pair + i
                nc.vector.tensor_copy(o_raw[:, h, qc, :], o_ps[i][0:D, :])  # stage; frees PSUM
                # denom row partition-shifted 64 -> 0 (approx_fast recip is
                # only correct at base partition 0)
                nc.vector.tensor_copy(dens0[:, i, :], o_ps[i][D:D + 1, :])
            for i in range(2):
                nc.vector.reciprocal_approx_fast(drecf[:, i, :], dens0[:, i, :])
            nc.vector.tensor_copy(drecb[:], drecf[:])
            for i in range(2):
                h = 2 * pair + i
                bc = pj.tile([D, 512], F32, tag="pj", name=f"bc{i}")
                nc.tensor.matmul(bc[:], sel0[:, 0:D], drecb[:, i, :],
                                 start=True, stop=True)
                nc.vector.tensor_tensor(o_all[i * D:(i + 1) * D, pair, qc * 512:(qc + 1) * 512],
                                        o_raw[:, h, qc, :], bc[:], mybir.AluOpType.mult)
        return norm

    wo_t = wpool.tile([P, MT, E], BF16, tag="w")

    def emit_wo():
        nc.scalar.dma_start(wo_t[:].rearrange("p a b -> p (a b)"), wo[:])
    fillers.append((34, 30, emit_wo))
    fillers.sort(key=lambda t: t[0])

    def emit_opart(st8):
        # pairs 0,1 + bias accumulated into SBUF; pair 2 lands later
        op1 = pj.tile([P, 512], F32, tag="pj", name="op1")
        op2 = pj.tile([P, 256], F32, tag="pj", name="op2")
        for pair in (0, 1):
            first, last = (pair == 0), (pair == 1)
            nc.tensor.matmul(op1[:], o_all[:, pair, st8 * P:(st8 + 1) * P],
                             wo_t[:, pair, 0:512], start=first, stop=last)
            nc.tensor.matmul(op2[:], o_all[:, pair, st8 * P:(st8 + 1) * P],
                             wo_t[:, pair, 512:768], start=first, stop=last)
        nc.vector.tensor_tensor(opart[:, st8, 0:512], op1[:], bo_sb[:, 0:512],
                                mybir.AluOpType.add)
        nc.vector.tensor_tensor(opart[:, st8, 512:768], op2[:], bo_sb[:, 512:768],
                                mybir.AluOpType.add)

    def emit_ofin(st8, pool=None):
        pool = pool or pj
        tg = "pj" if pool is pj else "o"
        op1 = pool.tile([P, 512], F32, tag=tg, name="op1")
        op2 = pool.tile([P, 256], F32, tag=tg, name="op2")
        nc.tensor.matmul(op1[:], o_all[:, 2, st8 * P:(st8 + 1) * P],
                         wo_t[:, 2, 0:512], start=True, stop=True)
        nc.tensor.matmul(op2[:], o_all[:, 2, st8 * P:(st8 + 1) * P],
                         wo_t[:, 2, 512:768], start=True, stop=True)
        out_sb = outpool.tile([P, E], F32, tag="osb")
        nc.vector.tensor_tensor(out_sb[:, 0:512], opart[:, st8, 0:512], op1[:],
                                mybir.AluOpType.add)
        nc.vector.tensor_tensor(out_sb[:, 512:768], opart[:, st8, 512:768], op2[:],
                                mybir.AluOpType.add)
        eng = (nc.sync, nc.scalar, nc.gpsimd)[st8 % 3]
        eng.dma_start(out[st8 * P:(st8 + 1) * P, :], out_sb[:])

    PVT = 4
    units = [(0, 0), (0, 1), (1, 0), (1, 1), (2, 0), (2, 1)]  # (pair, qc)
    # partials (pairs 0,1) during u4; pair-2 finishes for qc0 during u5
    opart_at = {(3, 6): 0, (3, 9): 1, (3, 12): 2, (3, 15): 3,
                (4, 8): 4, (4, 10): 5, (4, 12): 6, (4, 14): 7}
    ofin_at = {(5, 6): 0, (5, 8): 1, (5, 10): 2, (5, 12): 3}
    pending_norm = None
    pending_flush = []

    def flush_one():
        for _ in range(2):
            if pending_flush:
                pending_flush.pop(0)()

    for u, (pair, qc) in enumerate(units):
        o_ps = {i: opool.tile([D + 1, 512], F32, tag="o", name=f"o{i}") for i in range(2)}
        exq = []
        for kt in range(KT):
            drain_fillers(u * KT + kt, 2)
            flush_one()
            if kt == 6 and pending_norm is not None:
                pending_norm()
                pending_norm = None
            if (u, kt) in opart_at:
                emit_opart(opart_at[(u, kt)])
            if (u, kt) in ofin_at:
                emit_ofin(ofin_at[(u, kt)])
            st = psC.tile([P, 2, 512], F32, tag="sc")
            for i in range(2):
                po = D * i
                nc.tensor.matmul(st[:, i, :],
                                 kT[po:po + D, pair, kt * P:(kt + 1) * P],
                                 qT[po:po + D, pair, qc * 512:(qc + 1) * 512],
                                 start=True, stop=True)
            ex = epool.tile([P, 2, 512], BF16, tag="ex")
            nc.scalar.activation(ex[:, :, :], st[:, :, :], mybir.ActivationFunctionType.Exp)
            exq.append(ex)
            trail = PVT if u < len(units) - 1 else 0
            if kt >= trail:
                kk = kt - trail
                for i in range(2):
                    nc.tensor.matmul(o_ps[i][:, :], v_sb[:, kk, 2 * pair + i, :],
                                     exq[kk][:, i, :], start=(kk == 0),
                                     stop=(kk == KT - 1))

        def make_flush(pair, qc, o_ps, exq, kk):
            def f():
                for i in range(2):
                    nc.tensor.matmul(o_ps[i][:, :], v_sb[:, kk, 2 * pair + i, :],
                                     exq[kk][:, i, :], start=False, stop=(kk == KT - 1))
            return f
        if u < len(units) - 1:
            pending_flush = [make_flush(pair, qc, o_ps, exq, kk)
                             for kk in range(KT - PVT, KT)]
            pending_norm = make_norm(pair, qc, o_ps)
    last_ops = o_ps

    if dbg is not None:
        nc.sync.dma_start(dbg[:, :], o_raw[:].rearrange("p a b c -> p (a b c)"))

    lp, lqc = units[-1]
    for i in range(2):
        nc.vector.tensor_copy(dens0[:, i, :], last_ops[i][D:D + 1, :])
    for i in range(2):
        nc.vector.reciprocal_approx_fast(drecf[:, i, :], dens0[:, i, :])
    nc.vector.tensor_copy(drecb[:], drecf[:])
    for i in range(2):
        bc = pj.tile([D, 512], F32, tag="pj", name=f"bct{i}")
        nc.tensor.matmul(bc[:], sel0[:, 0:D], drecb[:, i, :], start=True, stop=True)
        nc.scalar.copy(bcl_sb[:, i, :], bc[:])
        nc.vector.tensor_tensor(o_all[i * D:(i + 1) * D, lp, lqc * 512:(lqc + 1) * 512],
                                last_ops[i][0:D, :], bcl_sb[:, i, :],
                                mybir.AluOpType.mult)
    for st8 in range(4, QB // P):
        emit_ofin(st8, pool=(pj if st8 % 2 == 0 else opool))


_NC_CACHE = None


def _get_nc():
    global _NC_CACHE
    if _NC_CACHE is None:
        _NC_CACHE = build_nc()
    return _NC_CACHE


def make_in_maps(query, key_, value, Wq, bq, Wk, bk, Wv, bv, Wo, bo):
    """Host-side sharding + layout prep. Returns list of 8 input dicts."""
    import ml_dtypes
    BF = ml_dtypes.bfloat16
    query = np.asarray(query, dtype=np.float32)
    key_ = np.asarray(key_, dtype=np.float32)
    value = np.asarray(value, dtype=np.float32)
    scale = np.float32(1.0 / np.sqrt(np.float32(D)))

    Wq = np.asarray(Wq, np.float32)
    Wk = np.asarray(Wk, np.float32)
    Wv = np.asarray(Wv, np.float32)
    Wo = np.asarray(Wo, np.float32)
    bq_f = np.asarray(bq, np.float32)
    bk_f = np.asarray(bk, np.float32)
    bv_f = np.asarray(bv, np.float32)
    bo_f = np.asarray(bo, np.float32)

    def pem(a):
        # [E, M] -> [128p, EC, M] -> flat [128, EC*M]
        E_, m = a.shape
        return np.ascontiguousarray(a.reshape(EC, P, m).transpose(1, 0, 2).reshape(P, EC * m))

    def xslices(a, width):
        # [E, S] -> [S//width, 128, EC*width]
        E_, s = a.shape
        n = s // width
        r = a.reshape(EC, P, n, width).transpose(2, 1, 0, 3)
        return np.ascontiguousarray(r.reshape(n, P, EC * width))

    xk_t = [xslices(key_[b].T, 512).astype(BF) for b in range(B)]
    xv_t = [xslices(value[b].T, P).astype(BF) for b in range(B)]
    xq_t = {}
    for b in range(B):
        for qh in range(2):
            xq_t[(b, qh)] = xslices(query[b, qh * QB:(qh + 1) * QB, :].T, 512).astype(BF)

    per_hh = {}
    for hh in range(2):
        hs = slice(hh * HL, (hh + 1) * HL)
        wq_f = np.transpose(Wq[hs], (1, 0, 2)).reshape(E, HL * D) * scale
        wk_f = np.transpose(Wk[hs], (1, 0, 2)).reshape(E, HL * D)
        wv_f = np.transpose(Wv[hs], (1, 0, 2)).reshape(E, HL * D)
        wo_f = Wo[hh * HL * D:(hh + 1) * HL * D, :]
        # wq/wk: per-m-tile chunks [MT, 128, EC*128]
        wq_c = np.stack([pem(wq_f[:, mt * P:(mt + 1) * P]) for mt in range(MT)]).astype(BF)
        wk_c = np.stack([pem(wk_f[:, mt * P:(mt + 1) * P]) for mt in range(MT)]).astype(BF)
        wv_c = pem(wv_f).astype(BF)
        # wo: [384, 768] -> [128, MT*768], partition p holds row mt*128+p
        wo_c = np.ascontiguousarray(
            wo_f.reshape(MT, P, E).transpose(1, 0, 2).reshape(P, MT * E)).astype(BF)
        bq_p = (bq_f[hs].reshape(HL * D) * scale).reshape(MT, P).T.copy()
        bk_p = bk_f[hs].reshape(HL * D).reshape(MT, P).T.copy()
        # v-bias folded through this core's Wo rows; bo itself only on hh=0
        bo_eff = bv_f[hs].reshape(HL * D) @ wo_f
        if hh == 0:
            bo_eff = bo_eff + bo_f
        per_hh[hh] = dict(
            wq0=wq_c[0], wqr=np.ascontiguousarray(wq_c[1:]),
            wk0=wk_c[0], wkr=np.ascontiguousarray(wk_c[1:]),
            wv=wv_c, wo=wo_c, bq=bq_p, bk=bk_p,
            bo=np.tile(bo_eff.reshape(1, E), (P, 1)).astype(np.float32).copy(),
        )

    in_maps = []
    for core in range(NCORES):
        b, qh, hh = core // 4, (core // 2) % 2, core % 2
        p = per_hh[hh]
        m = dict(
            qhead=np.ascontiguousarray(np.concatenate([p["wq0"], xq_t[(b, qh)][0]], axis=1)),
            khead=np.ascontiguousarray(np.concatenate([p["wk0"], xk_t[b][0]], axis=1)),
            xq1=xq_t[(b, qh)][1],
            xkr=np.ascontiguousarray(xk_t[b][1:]),
            wqr=p["wqr"], wkr=p["wkr"], wv=p["wv"], wo=p["wo"],
            bq=p["bq"], bk=p["bk"], bo=p["bo"],
            xv=xv_t[b],
        )
        in_maps.append(m)
    return in_maps


def assemble(results):
    outp = np.empty((B, S, E), dtype=np.float32)
    for b in range(B):
        for qh in range(2):
            c0 = b * 4 + qh * 2
            outp[b, qh * QB:(qh + 1) * QB, :] = results[c0]["out"] + results[c0 + 1]["out"]
    return outp


def kernel(query, key_, value, Wq, bq, Wk, bk, Wv, bv, Wo, bo):
    nc = _get_nc()
    in_maps = make_in_maps(query, key_, value, Wq, bq, Wk, bk, Wv, bv, Wo, bo)
    res = run_bass_kernel_spmd(nc, in_maps, core_ids=list(range(NCORES)))
    return assemble(res.results)


# revision 44
# speedup vs baseline: 1.1756x; 1.0133x over previous
# Multi-head attention kernel for Trainium2, sharded over 8 NeuronCores.
#
# Sharding: core = (batch b, query-half qh, head-half hh). Each core handles
# 6 heads (3 head-pairs) x 1024 queries of one batch, computing K/V
# projections only for its own 6 heads (no cross-core recompute). The output
# projection is a PARTIAL sum over the core's 6 heads; the two head-half
# partners' partials are summed on the HOST during assembly (exact fp32 add),
# so no on-chip collective is needed.
#
# The per-core kernel is built around the ScalarE exp stream (96 x
# [128,1024] activations, ~107us), the single largest engine span; all PE
# matmul work (~110us) is software-pipelined underneath it:
#   - attention runs as 6 units (head-pair x query-512-chunk) x 16 key
#     tiles: scores (2 heads row-packed in PE quadrants, K=64) -> exp ->
#     PV with a ones-column in v producing softmax denominators for free.
#     PV trails the exp stream by PVT steps so projection fillers never
#     delay the next scores.
#   - q/k/v projections are emitted as (deadline, not_before) filler chunks
#     drained into the attention steps; inputs stream via three DMA queues
#     (sync/scalar/gpsimd) in need order, with the first [wq|xq] / [wk|xk]
#     slices host-concatenated into single large "head" DMAs.
#   - per-unit softmax normalization (deferred into the next unit): staging
#     copy, denominators partition-shifted 64->0, single-op approx fast
#     reciprocal (only correct at partition 0!), bf16 K=1 broadcast matmul,
#     DVE multiply.
#   - output projection: pairs 0+1 and bias pre-accumulated into SBUF
#     during later units; only pair 2's two matmuls + one add + DMA per
#     128-query tile remain at the end.
# PSUM budget: scores 2x[128,2,512] (4 banks) + PV accumulators 2x[65,512]
# (2 banks) + projection scratch 2x[128,512] (2 banks) = 8 banks exactly.

import numpy as np
import os
from contextlib import ExitStack

_DEBUG = os.environ.get('KDEBUG', '0') == '1'

import concourse.bass as bass
import concourse.mybir as mybir
import concourse.tile as tile
from concourse import bacc
from concourse.bass_utils import run_bass_kernel_spmd

F32 = mybir.dt.float32
BF16 = mybir.dt.bfloat16
P = 128
E = 768
S = 2048
B = 2
H = 12
D = 64
NCORES = 8
EC = E // P        # 6 e-chunks (contraction over hidden)
KT = S // P        # 16 key tiles
HL = 6             # heads per core
MT = HL * D // P   # 3 m-tiles (head pairs) per core
QB = 1024          # queries per core
QC = 2             # query 512-chunks per core
NC4 = S // 512     # 4 n-slices of k^T


def build_nc():
    nc = bacc.Bacc("TRN2", debug=False)

    # DRAM I/O (per-core shapes; same NEFF on all 8 cores)
    # all inputs pre-arranged on host into on-chip [128-partition, ...] layout
    # so every DMA is a contiguous full-bandwidth copy
    # critical-path bundles: [wq-mt0 | xq-qc0] and [wk-mt0 | xk-n0] land as
    # single large DMAs so the exp stream can start ASAP
    qhead = nc.dram_tensor("qhead", (P, EC * P + EC * 512), BF16, kind="ExternalInput")
    khead = nc.dram_tensor("khead", (P, EC * P + EC * 512), BF16, kind="ExternalInput")
    xq1 = nc.dram_tensor("xq1", (P, EC * 512), BF16, kind="ExternalInput")
    xkr = nc.dram_tensor("xkr", (NC4 - 1, P, EC * 512), BF16, kind="ExternalInput")
    xv = nc.dram_tensor("xv", (KT, P, EC * P), BF16, kind="ExternalInput")
    wqr = nc.dram_tensor("wqr", (MT - 1, P, EC * P), BF16, kind="ExternalInput")
    wkr = nc.dram_tensor("wkr", (MT - 1, P, EC * P), BF16, kind="ExternalInput")
    wv = nc.dram_tensor("wv", (P, EC * HL * D), BF16, kind="ExternalInput")
    wo = nc.dram_tensor("wo", (P, MT * E), BF16, kind="ExternalInput")
    bq = nc.dram_tensor("bq", (P, MT), F32, kind="ExternalInput")     # per-partition bias per m-tile
    bk = nc.dram_tensor("bk", (P, MT), F32, kind="ExternalInput")
    bo = nc.dram_tensor("bo", (P, E), F32, kind="ExternalInput")      # partial (bv@Wo [+ bo]), broadcast
    out = nc.dram_tensor("out", (QB, E), F32, kind="ExternalOutput")  # PARTIAL over this core's heads
    dbg = None
    if _DEBUG:
        dbg = nc.dram_tensor("dbg", (D, HL * QC * 512), F32, kind="ExternalOutput")

    with tile.TileContext(nc) as tc:
        with ExitStack() as ctx:
            _emit(ctx, tc, nc, qhead, khead, xq1, xkr, xv, wqr, wkr, wv, wo,
                  bq, bk, bo, out, dbg)
    nc.compile()
    return nc


def _emit(ctx, tc, nc, qhead, khead, xq1, xkr, xv, wqr, wkr, wv, wo,
          bq, bk, bo, out, dbg=None):
    # ---- pools ----
    persist = ctx.enter_context(tc.tile_pool(name="persist", bufs=1))
    wpool = ctx.enter_context(tc.tile_pool(name="wpool", bufs=2))
    xvpool = ctx.enter_context(tc.tile_pool(name="xvpool", bufs=3))
    epool = ctx.enter_context(tc.tile_pool(name="epool", bufs=8))
    outpool = ctx.enter_context(tc.tile_pool(name="outpool", bufs=2))
    # PSUM pools: 4 + 2 + 2 = 8 banks
    psC = ctx.enter_context(tc.tile_pool(name="psC", bufs=2, space="PSUM"))    # scores [128,2,512]
    opool = ctx.enter_context(tc.tile_pool(name="opool", bufs=2, space="PSUM"))  # PV accum [65,512]
    pj = ctx.enter_context(tc.tile_pool(name="pj", bufs=2, space="PSUM"))      # proj scratch [128,512]

    # ---- persistent SBUF tensors ----
    qT = persist.tile([P, MT, QB], BF16)          # q^T [384, 1024]
    kT = persist.tile([P, MT, S], BF16)           # k^T [384, 2048]
    v_sb = persist.tile([P, KT, HL, D + 1], BF16)  # v + ones column per head
    o_all = persist.tile([P, MT, QB], BF16)       # normalized o^T, pairs in partition halves
    o_raw = persist.tile([D, HL, QC, 512], F32)   # staged unnormalized o^T
    dens0 = persist.tile([1, 2, 512], F32)        # denominators relocated to partition 0
    drecf = persist.tile([1, 2, 512], F32)        # fast-reciprocal output (partition 0)
    drecb = persist.tile([1, 2, 512], BF16)       # bf16 copy feeding the bc matmul
    sel0 = persist.tile([1, 512], BF16)           # ones row: bc selector + PE warm-up feed
    opart = persist.tile([P, QB // P, E], F32)    # pair0+pair1+bias partial out
    bcl_sb = persist.tile([D, 2, 512], F32)       # tail broadcast staged via ScalarE
    bq_sb = persist.tile([P, MT], F32)
    bk_sb = persist.tile([P, MT], F32)
    bo_sb = persist.tile([P, E], F32)
    qh_t = persist.tile([P, EC * P + EC * 512], BF16)   # wq-mt0 | xq-qc0
    kh_t = persist.tile([P, EC * P + EC * 512], BF16)   # wk-mt0 | xk-n0
    xq1_t = persist.tile([P, EC, 512], BF16)
    xkr_t = persist.tile([P, NC4 - 1, EC, 512], BF16)
    wqr_t = persist.tile([P, MT - 1, EC, P], BF16)
    wkr_t = persist.tile([P, MT - 1, EC, P], BF16)
    WOFF = EC * P

    # ---- DMAs: sync carries the q-chain bundle + late keys, scalar the
    # k-chain bundle + wv + early xv, gpsimd only small/late-needed items.
    nc.sync.dma_start(qh_t[:], qhead[:])
    nc.scalar.dma_start(kh_t[:], khead[:])
    wv_t = wpool.tile([P, EC, HL * D], BF16, tag="w")
    nc.gpsimd.dma_start(wv_t[:].rearrange("p a b -> p (a b)"), wv[:])
    nc.gpsimd.dma_start(bq_sb[:], bq[:])
    nc.gpsimd.dma_start(bk_sb[:], bk[:])
    nc.sync.dma_start(xkr_t[:, 0, :, :].rearrange("p a b -> p (a b)"), xkr[0])
    nc.scalar.dma_start(xkr_t[:, 1, :, :].rearrange("p a b -> p (a b)"), xkr[1])
    nc.scalar.dma_start(xkr_t[:, 2, :, :].rearrange("p a b -> p (a b)"), xkr[2])
    for mt in range(1, MT):
        nc.gpsimd.dma_start(wqr_t[:, mt - 1, :, :].rearrange("p a b -> p (a b)"), wqr[mt - 1])
        nc.gpsimd.dma_start(wkr_t[:, mt - 1, :, :].rearrange("p a b -> p (a b)"), wkr[mt - 1])

    def emit_xq1():
        nc.sync.dma_start(xq1_t[:].rearrange("p a b -> p (a b)"), xq1[:])

    def emit_bo():
        nc.sync.dma_start(bo_sb[:], bo[:])

    # constants: ones column for denominators, selector row for broadcast
    nc.vector.memset(v_sb[:, :, :, D], 1.0)
    nc.vector.memset(sel0[:], 1.0)

    # ---- PE warm-up: ~5us of dummy matmuls on memset data while the first
    # input DMAs are in flight, so the HAM clock-gate reaches 2.4GHz before
    # real work lands (cold MMs run at 1.2GHz otherwise)
    warm = pj.tile([D, 512], F32, tag="pj", name="warm")
    for i in range(12):
        nc.tensor.matmul(warm[:], sel0[:, 0:D], sel0[:],
                         start=(i == 0), stop=(i == 11))
    nc.vector.tensor_copy(o_raw[0:D, 0, 0, :], warm[:])  # consume (overwritten later)

    # ---- projection emitters (interleaved as filler work) ----
    def emit_q(mt, qc):
        ps = pj.tile([P, 512], F32, tag="pj")
        for ec in range(EC):
            lhsT = qh_t[:, ec * P:(ec + 1) * P] if mt == 0 else wqr_t[:, mt - 1, ec, :]
            rhs = (qh_t[:, WOFF + ec * 512:WOFF + (ec + 1) * 512] if qc == 0
                   else xq1_t[:, ec, :])
            nc.tensor.matmul(ps[:], lhsT, rhs, start=(ec == 0), stop=(ec == EC - 1))
        nc.vector.tensor_scalar_add(qT[:, mt, qc * 512:(qc + 1) * 512], ps[:], bq_sb[:, mt:mt + 1])

    def emit_k(mt, n4):
        ps = pj.tile([P, 512], F32, tag="pj")
        for ec in range(EC):
            lhsT = kh_t[:, ec * P:(ec + 1) * P] if mt == 0 else wkr_t[:, mt - 1, ec, :]
            rhs = (kh_t[:, WOFF + ec * 512:WOFF + (ec + 1) * 512] if n4 == 0
                   else xkr_t[:, n4 - 1, ec, :])
            nc.tensor.matmul(ps[:], lhsT, rhs, start=(ec == 0), stop=(ec == EC - 1))
        nc.vector.tensor_scalar_add(kT[:, mt, n4 * 512:(n4 + 1) * 512], ps[:], bk_sb[:, mt:mt + 1])

    xv_tiles = {}

    def emit_v_dma(kt):
        xv_t = xvpool.tile([P, EC, P], BF16, tag="xv")
        eng = nc.scalar if 6 <= kt < 12 else nc.sync
        eng.dma_start(xv_t[:].rearrange("p a b -> p (a b)"), xv[kt])
        xv_tiles[kt] = xv_t

    def emit_v(kt):
        xv_t = xv_tiles.pop(kt)
        ps = pj.tile([P, 512], F32, tag="pj")  # only 384 used
        for ec in range(EC):
            nc.tensor.matmul(ps[:, 0:HL * D], xv_t[:, ec, :], wv_t[:, ec, :],
                             start=(ec == 0), stop=(ec == EC - 1))
        nc.vector.tensor_copy(v_sb[:, kt, :, 0:D],
                              ps[:, 0:HL * D].rearrange("p (h d) -> p h d", d=D))

    # Deadline-ordered filler queue: (deadline_step, emit_fn). Steps run
    # 0..95 (6 units x 16 key tiles). Forced at deadline; otherwise drained
    # at DRAIN_BUDGET items/step to spread PE work under the exp stream.
    fillers = []
    # (deadline, not_before, fn): forced at deadline; budget-drained only
    # once `not_before` is reached (so PE work never lands ahead of its DMA)
    dma_dl = {0: 0, 1: 0, 2: 1, 3: 2, 4: 3, 5: 4, 6: 5, 7: 5,
              8: 7, 9: 8, 10: 9, 11: 10, 12: 11, 13: 12, 14: 13, 15: 14}
    for kt in range(KT):
        fillers.append((dma_dl[kt], 0, lambda kt=kt: emit_v_dma(kt)))
    fillers.append((6, 6, emit_xq1))
    fillers.append((40, 36, emit_bo))
    for n4 in range(1, NC4):
        fillers.append((4 * n4 - 1, 4 * n4 - 2, lambda n4=n4: emit_k(0, n4)))
    # v-projection MMs: must be emitted before PV(unit0, kt), which trails
    # the exp stream by PVT steps (Tile deps follow emission order)
    for kt in range(KT):
        fillers.append((max(kt + 3, 4), max(kt + 1, 4), lambda kt=kt: emit_v(kt)))
    # pair-major unit order: u1=(p0,qc1)@16, u2=(p1,qc0)@32, u4=(p2,qc0)@64
    fillers.append((12, 8, lambda: emit_q(0, 1)))
    fillers.append((26, 20, lambda: emit_q(1, 0)))
    for n4 in range(NC4):
        fillers.append((27 + n4, 21 + n4, lambda n4=n4: emit_k(1, n4)))
    fillers.append((44, 40, lambda: emit_q(1, 1)))
    fillers.append((56, 52, lambda: emit_q(2, 0)))
    for n4 in range(NC4):
        fillers.append((57 + n4, 53 + n4, lambda n4=n4: emit_k(2, n4)))
    fillers.append((74, 70, lambda: emit_q(2, 1)))
    fillers.sort(key=lambda t: t[0])
    fidx = [0]

    def drain_fillers(step, budget):
        n = 0
        while fidx[0] < len(fillers) and (
                fillers[fidx[0]][0] <= step
                or (n < budget and fillers[fidx[0]][1] <= step)):
            fillers[fidx[0]][2]()
            fidx[0] += 1
            n += 1

    # ---- prologue: minimal work before the exp stream starts ----
    emit_q(0, 0)
    emit_k(0, 0)

    # ---- attention units: qc-major (all qc0 pairs first) so the qc0
    # output projection can run interleaved into the qc1 units.
    # PV trails the exp stream by PVT steps so v-projection fillers keep
    # lower PE priority than scores and the exp cadence never breaks.
    def make_norm(pair, qc, o_ps):
        def norm():
            for i in range(2):
                h = 2 * pair + i
                nc.vector.tensor_copy(o_raw[:, h, qc, :], o_ps[i][0:D, :])  # stage; frees PSUM
                # denom row partition-shifted 64 -> 0 (approx_fast recip is
                # only correct at base partition 0)
                nc.vector.tensor_copy(dens0[:, i, :], o_ps[i][D:D + 1, :])
            for i in range(2):
                nc.vector.reciprocal_approx_fast(drecf[:, i, :], dens0[:, i, :])
            nc.vector.tensor_copy(drecb[:], drecf[:])
            for i in range(2):
                h = 2 * pair + i
                bc = pj.tile([D, 512], F32, tag="pj", name=f"bc{i}")
                nc.tensor.matmul(bc[:], sel0[:, 0:D], drecb[:, i, :],
                                 start=True, stop=True)
                nc.vector.tensor_tensor(o_all[i * D:(i + 1) * D, pair, qc * 512:(qc + 1) * 512],
                                        o_raw[:, h, qc, :], bc[:], mybir.AluOpType.mult)
        return norm

    wo_t = wpool.tile([P, MT, E], BF16, tag="w")

    def emit_wo():
        nc.scalar.dma_start(wo_t[:].rearrange("p a b -> p (a b)"), wo[:])
    fillers.append((34, 30, emit_wo))
    fillers.sort(key=lambda t: t[0])

    def emit_opart(st8):
        # pairs 0,1 + bias accumulated into SBUF; pair 2 lands later
        op1 = pj.tile([P, 512], F32, tag="pj", name="op1")
        op2 = pj.tile([P, 256], F32, tag="pj", name="op2")
        for pair in (0, 1):
            first, last = (pair == 0), (pair == 1)
            nc.tensor.matmul(op1[:], o_all[:, pair, st8 * P:(st8 + 1) * P],
                             wo_t[:, pair, 0:512], start=first, stop=last)
            nc.tensor.matmul(op2[:], o_all[:, pair, st8 * P:(st8 + 1) * P],
                             wo_t[:, pair, 512:768], start=first, stop=last)
        nc.vector.tensor_tensor(opart[:, st8, 0:512], op1[:], bo_sb[:, 0:512],
                                mybir.AluOpType.add)
        nc.vector.tensor_tensor(opart[:, st8, 512:768], op2[:], bo_sb[:, 512:768],
                                mybir.AluOpType.add)

    def emit_ofin(st8, pool=None):
        pool = pool or pj
        tg = "pj" if pool is pj else "o"
        op1 = pool.tile([P, 512], F32, tag=tg, name="op1")
        op2 = pool.tile([P, 256], F32, tag=tg, name="op2")
        nc.tensor.matmul(op1[:], o_all[:, 2, st8 * P:(st8 + 1) * P],
                         wo_t[:, 2, 0:512], start=True, stop=True)
        nc.tensor.matmul(op2[:], o_all[:, 2, st8 * P:(st8 + 1) * P],
                         wo_t[:, 2, 512:768], start=True, stop=True)
        out_sb = outpool.tile([P, E], F32, tag="osb")
        nc.vector.tensor_tensor(out_sb[:, 0:512], opart[:, st8, 0:512], op1[:],
                                mybir.AluOpType.add)
        nc.vector.tensor_tensor(out_sb[:, 512:768], opart[:, st8, 512:768], op2[:],
                                mybir.AluOpType.add)
        eng = (nc.sync, nc.scalar, nc.gpsimd)[st8 % 3]
        eng.dma_start(out[st8 * P:(st8 + 1) * P, :], out_sb[:])

    PVT = 4
    units = [(0, 0), (0, 1), (1, 0), (1, 1), (2, 0), (2, 1)]  # (pair, qc)
    # partials (pairs 0,1) during u4; pair-2 finishes for qc0 during u5
    opart_at = {(3, 6): 0, (3, 9): 1, (3, 12): 2, (3, 15): 3,
                (4, 8): 4, (4, 10): 5, (4, 12): 6, (4, 14): 7}
    ofin_at = {(5, 6): 0, (5, 8): 1, (5, 10): 2, (5, 12): 3}
    pending_norm = None
    pending_flush = []

    def flush_one():
        for _ in range(2):
            if pending_flush:
                pending_flush.pop(0)()

    for u, (pair, qc) in enumerate(units):
        o_ps = {i: opool.tile([D + 1, 512], F32, tag="o", name=f"o{i}") for i in range(2)}
        exq = []
        for kt in range(KT):
            drain_fillers(u * KT + kt, 2)
            flush_one()
            if kt == 6 and pending_norm is not None:
                pending_norm()
                pending_norm = None
            if (u, kt) in opart_at:
                emit_opart(opart_at[(u, kt)])
            if (u, kt) in ofin_at:
                emit_ofin(ofin_at[(u, kt)])
            st = psC.tile([P, 2, 512], F32, tag="sc")
            for i in range(2):
                po = D * i
                nc.tensor.matmul(st[:, i, :],
                                 kT[po:po + D, pair, kt * P:(kt + 1) * P],
                                 qT[po:po + D, pair, qc * 512:(qc + 1) * 512],
                                 start=True, stop=True)
            ex = epool.tile([P, 2, 512], BF16, tag="ex")
            nc.scalar.activation(ex[:, :, :], st[:, :, :], mybir.ActivationFunctionType.Exp)
            exq.append(ex)
            trail = PVT if u < len(units) - 1 else 0
            if kt >= trail:
                kk = kt - trail
                for i in range(2):
                    nc.tensor.matmul(o_ps[i][:, :], v_sb[:, kk, 2 * pair + i, :],
                                     exq[kk][:, i, :], start=(kk == 0),
                                     stop=(kk == KT - 1))

        def make_flush(pair, qc, o_ps, exq, kk):
            def f():
                for i in range(2):
                    nc.tensor.matmul(o_ps[i][:, :], v_sb[:, kk, 2 * pair + i, :],
                                     exq[kk][:, i, :], start=False, stop=(kk == KT - 1))
            return f
        if u < len(units) - 1:
            pending_flush = [make_flush(pair, qc, o_ps, exq, kk)
                             for kk in range(KT - PVT, KT)]
            pending_norm = make_norm(pair, qc, o_ps)
    last_ops = o_ps

    if dbg is not None:
        nc.sync.dma_start(dbg[:, :], o_raw[:].rearrange("p a b c -> p (a b c)"))

    lp, lqc = units[-1]
    for i in range(2):
        nc.vector.tensor_copy(dens0[:, i, :], last_ops[i][D:D + 1, :])
    for i in range(2):
        nc.vector.reciprocal_approx_fast(drecf[:, i, :], dens0[:, i, :])
    nc.vector.tensor_copy(drecb[:], drecf[:])
    for i in range(2):
        bc = pj.tile([D, 512], F32, tag="pj", name=f"bct{i}")
        nc.tensor.matmul(bc[:], sel0[:, 0:D], drecb[:, i, :], start=True, stop=True)
        nc.scalar.copy(bcl_sb[:, i, :], bc[:])
        nc.vector.tensor_tensor(o_all[i * D:(i + 1) * D, lp, lqc * 512:(lqc + 1) * 512],
                                last_ops[i][0:D, :], bcl_sb[:, i, :],
                                mybir.AluOpType.mult)
    # keep the PE clock warm into the tail: dummy matmuls gated on drecb so
    # the scheduler cannot hoist them earlier
    warm2 = pj.tile([D, 512], F32, tag="pj", name="warm2")
    for i in range(6):
        nc.tensor.matmul(warm2[:], sel0[:, 0:D], drecb[:, 0, :],
                         start=(i == 0), stop=(i == 5))
    nc.vector.tensor_copy(o_raw[0:D, 0, 0, :], warm2[:])
    for st8 in range(4, QB // P):
        emit_ofin(st8, pool=(pj if st8 % 2 == 0 else opool))


_NC_CACHE = None


def _get_nc():
    global _NC_CACHE
    if _NC_CACHE is None:
        _NC_CACHE = build_nc()
    return _NC_CACHE


def make_in_maps(query, key_, value, Wq, bq, Wk, bk, Wv, bv, Wo, bo):
    """Host-side sharding + layout prep. Returns list of 8 input dicts."""
    import ml_dtypes
    BF = ml_dtypes.bfloat16
    query = np.asarray(query, dtype=np.float32)
    key_ = np.asarray(key_, dtype=np.float32)
    value = np.asarray(value, dtype=np.float32)
    scale = np.float32(1.0 / np.sqrt(np.float32(D)))

    Wq = np.asarray(Wq, np.float32)
    Wk = np.asarray(Wk, np.float32)
    Wv = np.asarray(Wv, np.float32)
    Wo = np.asarray(Wo, np.float32)
    bq_f = np.asarray(bq, np.float32)
    bk_f = np.asarray(bk, np.float32)
    bv_f = np.asarray(bv, np.float32)
    bo_f = np.asarray(bo, np.float32)

    def pem(a):
        # [E, M] -> [128p, EC, M] -> flat [128, EC*M]
        E_, m = a.shape
        return np.ascontiguousarray(a.reshape(EC, P, m).transpose(1, 0, 2).reshape(P, EC * m))

    def xslices(a, width):
        # [E, S] -> [S//width, 128, EC*width]
        E_, s = a.shape
        n = s // width
        r = a.reshape(EC, P, n, width).transpose(2, 1, 0, 3)
        return np.ascontiguousarray(r.reshape(n, P, EC * width))

    xk_t = [xslices(key_[b].T, 512).astype(BF) for b in range(B)]
    xv_t = [xslices(value[b].T, P).astype(BF) for b in range(B)]
    xq_t = {}
    for b in range(B):
        for qh in range(2):
            xq_t[(b, qh)] = xslices(query[b, qh * QB:(qh + 1) * QB, :].T, 512).astype(BF)

    per_hh = {}
    for hh in range(2):
        hs = slice(hh * HL, (hh + 1) * HL)
        wq_f = np.transpose(Wq[hs], (1, 0, 2)).reshape(E, HL * D) * scale
        wk_f = np.transpose(Wk[hs], (1, 0, 2)).reshape(E, HL * D)
        wv_f = np.transpose(Wv[hs], (1, 0, 2)).reshape(E, HL * D)
        wo_f = Wo[hh * HL * D:(hh + 1) * HL * D, :]
        # wq/wk: per-m-tile chunks [MT, 128, EC*128]
        wq_c = np.stack([pem(wq_f[:, mt * P:(mt + 1) * P]) for mt in range(MT)]).astype(BF)
        wk_c = np.stack([pem(wk_f[:, mt * P:(mt + 1) * P]) for mt in range(MT)]).astype(BF)
        wv_c = pem(wv_f).astype(BF)
        # wo: [384, 768] -> [128, MT*768], partition p holds row mt*128+p
        wo_c = np.ascontiguousarray(
            wo_f.reshape(MT, P, E).transpose(1, 0, 2).reshape(P, MT * E)).astype(BF)
        bq_p = (bq_f[hs].reshape(HL * D) * scale).reshape(MT, P).T.copy()
        bk_p = bk_f[hs].reshape(HL * D).reshape(MT, P).T.copy()
        # v-bias folded through this core's Wo rows; bo itself only on hh=0
        bo_eff = bv_f[hs].reshape(HL * D) @ wo_f
        if hh == 0:
            bo_eff = bo_eff + bo_f
        per_hh[hh] = dict(
            wq0=wq_c[0], wqr=np.ascontiguousarray(wq_c[1:]),
            wk0=wk_c[0], wkr=np.ascontiguousarray(wk_c[1:]),
            wv=wv_c, wo=wo_c, bq=bq_p, bk=bk_p,
            bo=np.tile(bo_eff.reshape(1, E), (P, 1)).astype(np.float32).copy(),
        )

    in_maps = []
    for core in range(NCORES):
        b, qh, hh = core // 4, (core // 2) % 2, core % 2
        p = per_hh[hh]
        m = dict(
            qhead=np.ascontiguousarray(np.concatenate([p["wq0"], xq_t[(b, qh)][0]], axis=1)),
            khead=np.ascontiguousarray(np.concatenate([p["wk0"], xk_t[b][0]], axis=1)),
            xq1=xq_t[(b, qh)][1],
            xkr=np.ascontiguousarray(xk_t[b][1:]),
            wqr=p["wqr"], wkr=p["wkr"], wv=p["wv"], wo=p["wo"],
            bq=p["bq"], bk=p["bk"], bo=p["bo"],
            xv=xv_t[b],
        )
        in_maps.append(m)
    return in_maps


def assemble(results):
    outp = np.empty((B, S, E), dtype=np.float32)
    for b in range(B):
        for qh in range(2):
            c0 = b * 4 + qh * 2
            outp[b, qh * QB:(qh + 1) * QB, :] = results[c0]["out"] + results[c0 + 1]["out"]
    return outp


def kernel(query, key_, value, Wq, bq, Wk, bk, Wv, bv, Wo, bo):
    nc = _get_nc()
    in_maps = make_in_maps(query, key_, value, Wq, bq, Wk, bk, Wv, bv, Wo, bo)
    res = run_bass_kernel_spmd(nc, in_maps, core_ids=list(range(NCORES)))
    return assemble(res.results)
